# revision 1
# baseline (speedup 1.0000x reference)
"""Bahdanau attention kernel for Trainium2 (Bass/Tile), 8-core data-parallel.

Problem shapes: B=32, Tx=1024, enc_hid=dec_hid=attn=1024, fp32.

Math (per example b):
  dec_proj = W_dec @ dec_hidden[b]                 [attn]
  energy^T[a, t] = tanh(sum_e W_enc[a,e] enc[b,t,e] + dec_proj[a] + W_b[a])
  scores[t] = sum_a v[a] energy^T[a, t]
  alpha = softmax(mask(scores))
  context[e] = sum_t alpha[t] enc[b,t,e]

Sharding: batch B split 4 examples per core across 8 cores; weights replicated.
All big matmuls run as float32r (fp32 truncated to fp22 in the PE) which at
moving-dim >= 256 streams at full PE rate.

Layouts per core (host-side preprocessing in kernel()):
  encT  [4, E, Tx]   enc transposed  -> e on partitions (enc_proj rhs)
  enc   [4, Tx, E]   natural         -> t on partitions (context rhs)
  w_encT [E, A], w_decT [D, A]       transposed nn.Linear weights
  dec_hT [D, 4], v_col [A, 1], wb8 [128, 8], maskf [4, Tx]
"""

from contextlib import ExitStack

import numpy as np

import concourse.bass as bass
import concourse.tile as tile
from concourse import bacc, mybir
from concourse.masks import make_identity

F32 = mybir.dt.float32
F32R = mybir.dt.float32r
AF = mybir.ActivationFunctionType

P = 128
N_CORES = 8
B_LOC = 4            # examples per core
TX = 1024
E = 1024             # enc_hid
A = 1024             # attn
D = 1024             # dec_hid
EO = E // P          # e-chunks
AO = A // P          # a-chunks
TO = TX // P         # t-chunks
DO = D // P          # d-chunks
NT = TX // 512       # t-tiles for energy free dim
ET = E // 512        # e-tiles for context free dim


def _r(ap):
    """View an fp32 AP as float32r for full-rate PE streaming."""
    return ap.bitcast(F32R)


def build_nc():
    nc = bacc.Bacc(
        "TRN2", target_bir_lowering=False, debug=False, num_devices=N_CORES
    )
    encT = nc.dram_tensor("encT", [B_LOC, E, TX], F32, kind="ExternalInput").ap()
    enc = nc.dram_tensor("enc", [B_LOC, TX, E], F32, kind="ExternalInput").ap()
    w_encT = nc.dram_tensor("w_encT", [E, A], F32, kind="ExternalInput").ap()
    w_decT = nc.dram_tensor("w_decT", [D, A], F32, kind="ExternalInput").ap()
    dec_hT = nc.dram_tensor("dec_hT", [D, B_LOC], F32, kind="ExternalInput").ap()
    v_col = nc.dram_tensor("v_col", [A, 1], F32, kind="ExternalInput").ap()
    wb8 = nc.dram_tensor("wb8", [P, AO], F32, kind="ExternalInput").ap()
    maskf = nc.dram_tensor("maskf", [B_LOC, TX], F32, kind="ExternalInput").ap()
    ctx_out = nc.dram_tensor("context", [B_LOC, E], F32, kind="ExternalOutput").ap()
    alpha_out = nc.dram_tensor("alpha", [B_LOC, TX], F32, kind="ExternalOutput").ap()

    # Queue/lane discipline (each DMA-capable engine owns ONE FIFO queue):
    #   sync   (HWDGE ~200GB/s): all order-sensitive / slot-gated traffic.
    #   scalar (HWDGE ~200GB/s): ONLY dependency-free start-of-kernel loads --
    #          a gated DMA enqueue would head-of-line block the ACT compute
    #          stream (tanh/exp) and backpressure the PE.
    #   gpsimd (SWDGE ~130GB/s): small constants + the enc-natural stream.
    with tile.TileContext(nc) as tc, ExitStack() as ctx:
        const = ctx.enter_context(tc.tile_pool(name="const", bufs=1))
        big = ctx.enter_context(tc.tile_pool(name="big", bufs=4))
        en_pool = ctx.enter_context(tc.tile_pool(name="energy", bufs=6))
        small = ctx.enter_context(tc.tile_pool(name="small", bufs=2))
        rowp = ctx.enter_context(tc.tile_pool(name="rows", bufs=2))
        ep_psum = ctx.enter_context(tc.tile_pool(name="ep_ps", bufs=4, space="PSUM"))
        vec_psum = ctx.enter_context(tc.tile_pool(name="vec_ps", bufs=4, space="PSUM"))
        dram = ctx.enter_context(tc.tile_pool(name="dram", bufs=2, space="DRAM"))

        # ---- preamble loads. Aggregate DMA saturates at ~320GB/s across 3
        # concurrent queues (~107GB/s each), so every bulk stream is emitted
        # in NEED order round-robin across all three lanes; per-queue FIFO
        # then acts as the bandwidth arbiter. Order: w_decT a<512 halves
        # (dec_proj matmuls head the in-order PE stream and feed the bias
        # chain), then the w_encT/encT0 pairs that pace b=0's energy groups,
        # then w_decT a>=512, then the b=1 prefetch.
        w_encT_sb = const.tile([P, EO, A], F32R)
        encT_tiles = {}
        encT_tiles[0] = big.tile([P, EO, TX], F32R, tag="big", name="encT_sb0")
        encT_tiles[1] = big.tile([P, EO, TX], F32R, tag="big", name="encT_sb1")
        w_decT_sb = big.tile([P, DO, A], F32R, tag="big", name="w_decT_sb")

        dec_hT_sb = const.tile([P, DO, B_LOC], F32R)
        nc.gpsimd.dma_start(
            dec_hT_sb[:], dec_hT.rearrange("(do p) b -> p do b", p=P).bitcast(F32R)
        )
        v_sb = const.tile([P, AO, 1], F32R)
        nc.gpsimd.dma_start(
            v_sb[:], v_col.rearrange("(ao p) one -> p ao one", p=P).bitcast(F32R)
        )
        wb_sb = const.tile([P, AO], F32)
        nc.gpsimd.dma_start(wb_sb[:], wb8[:])
        mask_rows = []
        for b in range(B_LOC):
            mr = small.tile([1, TX], F32, tag="mrow", bufs=B_LOC, name=f"mask{b}")
            nc.gpsimd.dma_start(mr[:], maskf[b : b + 1, :])
            mask_rows.append(mr)
        ident4 = const.tile([B_LOC, B_LOC], F32)
        make_identity(nc, ident4[:])

        lanes = [nc.sync, nc.scalar, nc.gpsimd]
        lane_i = [0]

        def lane():
            eng = lanes[lane_i[0] % 3]
            lane_i[0] += 1
            return eng

        for do in range(DO):
            lane().dma_start(
                w_decT_sb[:, do, 0:512],
                w_decT[do * P : (do + 1) * P, 0:512].bitcast(F32R),
            )
        for eo in range(EO):
            lane().dma_start(
                w_encT_sb[:, eo], w_encT[eo * P : (eo + 1) * P, :].bitcast(F32R)
            )
            lane().dma_start(
                encT_tiles[0][:, eo], encT[0, eo * P : (eo + 1) * P, :].bitcast(F32R)
            )
        for do in range(DO):
            lane().dma_start(
                w_decT_sb[:, do, 512:1024],
                w_decT[do * P : (do + 1) * P, 512:1024].bitcast(F32R),
            )
        for eo in range(EO):
            lane().dma_start(
                encT_tiles[1][:, eo], encT[1, eo * P : (eo + 1) * P, :].bitcast(F32R)
            )
        bias_sb = const.tile([P, AO, B_LOC], F32)

        # ---- dec_proj: psum[4b, 512a] tiles, b-stationary f32r matmuls ----
        dp_row = rowp.tile([B_LOC, A], F32, tag="row4k", name="dp_row")
        for at in range(A // 512):
            dp_ps = ep_psum.tile([P, 512], F32, tag="ep", name=f"dp_ps{at}")
            for do in range(DO):
                nc.tensor.matmul(
                    dp_ps[:B_LOC, :],
                    lhsT=dec_hT_sb[:, do],
                    rhs=w_decT_sb[:, do, at * 512 : (at + 1) * 512],
                    start=(do == 0),
                    stop=(do == DO - 1),
                )
            nc.vector.tensor_copy(
                dp_row[:, at * 512 : (at + 1) * 512], dp_ps[:B_LOC, :]
            )

        def finalize_bias():
            # bias[a-part, b] = dec_proj^T + W_b via PE transposes (PE is
            # otherwise DMA-starved this early; no DRAM bounce needed).
            for ao in range(AO):
                tp_ps = vec_psum.tile(
                    [P, B_LOC], F32, tag="vec", name=f"tp_ps{ao}"
                )
                nc.tensor.transpose(
                    tp_ps[:], dp_row[:, ao * P : (ao + 1) * P], ident4[:]
                )
                nc.vector.tensor_scalar_add(
                    bias_sb[:, ao], tp_ps[:], wb_sb[:, ao : ao + 1]
                )

        finalize_bias()

        # ---- per-example pipeline -----------------------------------------
        for b in range(B_LOC):
            if b >= 2:
                encT_tiles[b] = big.tile(
                    [P, EO, TX], F32R, tag="big", name=f"encT_sb{b}"
                )
                for eo in range(EO):
                    lane().dma_start(
                        encT_tiles[b][:, eo],
                        encT[b, eo * P : (eo + 1) * P, :].bitcast(F32R),
                    )
            encT_sb = encT_tiles[b]
            # prefetch this example's natural-layout enc for the context step
            enc_nat = big.tile([P, TO, E], F32R, tag="big", name=f"encN_sb{b}")
            for to in range(TO):
                lane().dma_start(
                    enc_nat[:, to], enc[b, to * P : (to + 1) * P, :].bitcast(F32R)
                )
            mask_row = mask_rows[b]

            # energy^T tiles + score accumulation
            sc_ps = [
                vec_psum.tile([1, 512], F32, tag="vec", name=f"sc{b}_{nt}")
                for nt in range(NT)
            ]
            for ao in range(AO):
                for nt in range(NT):
                    ep_ps = ep_psum.tile(
                        [P, 512], F32, tag="ep", name=f"ep{b}_{ao}_{nt}"
                    )
                    for eo in range(EO):
                        nc.tensor.matmul(
                            ep_ps[:],
                            lhsT=w_encT_sb[:, eo, ao * P : (ao + 1) * P],
                            rhs=encT_sb[:, eo, nt * 512 : (nt + 1) * 512],
                            start=(eo == 0),
                            stop=(eo == EO - 1),
                        )
                    energy = en_pool.tile(
                        [P, 512], F32R, tag="energy", name=f"en{b}_{ao}_{nt}"
                    )
                    nc.scalar.activation(
                        energy[:], ep_ps[:], AF.Tanh, bias=bias_sb[:, ao, b : b + 1]
                    )
                    nc.tensor.matmul(
                        sc_ps[nt][:],
                        lhsT=v_sb[:, ao],
                        rhs=energy[:],
                        start=(ao == 0),
                        stop=(ao == AO - 1),
                    )

            # softmax with masking (all on partition 0). Scores are bounded
            # (|s| <= sum|v| ~ 26 since |tanh| <= 1) so exp needs no max
            # shift -- softmax is shift-invariant. exp reads score PSUM
            # directly; the exp -> mask -> DRAM-bounce transpose chain is
            # pipelined per 512-half so the first context matmuls overlap
            # the second half. (The bounce exists because sbuf->sbuf
            # partition-crossing reshapes don't balance as DMA APs.)
            exp_row = rowp.tile([1, TX], F32, tag="erow", name=f"exp{b}")
            exp_scr = dram.tile([TX], F32, tag="escr", name=f"escr{b}")
            expT = small.tile([P, TO], F32R, tag="expT", name=f"expT{b}")
            exp_scr_t = exp_scr.rearrange("(to p) -> p to", p=P).bitcast(F32R)
            HTO = TO // NT
            for nt in range(NT):
                hs = slice(nt * 512, (nt + 1) * 512)
                nc.scalar.activation(exp_row[:, hs], sc_ps[nt][:], AF.Exp)
                nc.vector.tensor_mul(
                    out=exp_row[:, hs], in0=exp_row[:, hs], in1=mask_row[:, hs]
                )
                nc.sync.dma_start(exp_scr[None, hs], exp_row[:, hs])
                nc.sync.dma_start(
                    expT[:, nt * HTO : (nt + 1) * HTO],
                    exp_scr_t[:, nt * HTO : (nt + 1) * HTO],
                )

            ssum = small.tile([1, 1], F32, tag="ssum", name=f"ssum{b}")
            nc.vector.reduce_sum(ssum[:], exp_row[:], axis=mybir.AxisListType.X)
            rsum = small.tile([1, 1], F32, tag="rsum", name=f"rsum{b}")
            nc.vector.reciprocal(rsum[:], ssum[:])
            # normalize in place (the expT bounce DMAs already read exp_row;
            # Tile orders the WAR dependency) and emit alpha
            nc.vector.tensor_scalar_mul(exp_row[:], exp_row[:], rsum[:])
            nc.sync.dma_start(alpha_out[b : b + 1, :], exp_row[:])

            # context[e] = sum_t alpha[t] enc[t, e]; normalization by 1/sum
            # is folded into the PSUM evacuation
            ctx_row = rowp.tile([1, E], F32, tag="row4k", name=f"ctx{b}")
            for et in range(ET):
                cx_ps = vec_psum.tile([1, 512], F32, tag="vec", name=f"cx{b}_{et}")
                for to in range(TO):
                    nc.tensor.matmul(
                        cx_ps[:],
                        lhsT=expT[:, to : to + 1],
                        rhs=enc_nat[:, to, et * 512 : (et + 1) * 512],
                        start=(to == 0),
                        stop=(to == TO - 1),
                    )
                nc.vector.tensor_scalar_mul(
                    ctx_row[:, et * 512 : (et + 1) * 512], cx_ps[:], rsum[:]
                )
            nc.sync.dma_start(ctx_out[b : b + 1, :], ctx_row[:])

    nc.compile()
    return nc


_NC = None


def _get_nc():
    global _NC
    if _NC is None:
        _NC = build_nc()
    return _NC


def make_in_maps(dec_hidden, enc_outputs, mask, W_w, W_b, v_w):
    dec_hidden = np.asarray(dec_hidden, np.float32)
    enc_outputs = np.asarray(enc_outputs, np.float32)
    W_w = np.asarray(W_w, np.float32)
    W_b = np.asarray(W_b, np.float32)
    v_w = np.asarray(v_w, np.float32)
    maskf = np.asarray(mask).astype(np.float32)

    enc = np.ascontiguousarray(enc_outputs)
    encT = np.ascontiguousarray(enc_outputs.transpose(0, 2, 1))
    w_encT = np.ascontiguousarray(W_w[:, D:].T)
    w_decT = np.ascontiguousarray(W_w[:, :D].T)
    wb8 = np.ascontiguousarray(W_b.reshape(AO, P).T)
    v_col = np.ascontiguousarray(v_w.reshape(A, 1))

    in_maps = []
    for c in range(N_CORES):
        sl = slice(B_LOC * c, B_LOC * (c + 1))
        in_maps.append(
            {
                "encT": encT[sl],
                "enc": enc[sl],
                "w_encT": w_encT,
                "w_decT": w_decT,
                "dec_hT": np.ascontiguousarray(dec_hidden[sl].T),
                "v_col": v_col,
                "wb8": wb8,
                "maskf": np.ascontiguousarray(maskf[sl]),
            }
        )
    return in_maps


def kernel(dec_hidden, enc_outputs, mask, W_w, W_b, v_w):
    from concourse.bass_utils import run_bass_kernel_spmd

    assert enc_outputs.shape == (N_CORES * B_LOC, TX, E), enc_outputs.shape
    nc = _get_nc()
    in_maps = make_in_maps(dec_hidden, enc_outputs, mask, W_w, W_b, v_w)
    res = run_bass_kernel_spmd(nc, in_maps, list(range(N_CORES))).results
    context = np.concatenate([res[c]["context"] for c in range(N_CORES)], axis=0)
    alpha = np.concatenate([res[c]["alpha"] for c in range(N_CORES)], axis=0)
    return context, alpha



# revision 4
# speedup vs baseline: 1.0438x; 1.0438x over previous
"""Bahdanau attention kernel for Trainium2 (Bass/Tile), 8-core data-parallel.

Problem shapes: B=32, Tx=1024, enc_hid=dec_hid=attn=1024, fp32 in/out.

Math (per example b):
  dec_proj = W_dec @ dec_hidden[b]                 [attn]
  energy^T[a, t] = tanh(sum_e W_enc[a,e] enc[b,t,e] + dec_proj[a] + W_b[a])
  scores[t] = sum_a v[a] energy^T[a, t]
  alpha = softmax(mask(scores))
  context[e] = sum_t alpha[t] enc[b,t,e]

Sharding: batch B split 4 examples per core across 8 cores; weights replicated.

All matmul operands are bf16: on TRN2 silicon a 512-moving-row fp32r matmul
measures ~394ns while bf16 measures ~216ns (1 col/cycle @2.4GHz), and bf16
halves HBM traffic (42MB -> 21MB per core) and enables fast weight load.
PSUM accumulation stays fp32. Measured fp22 end-to-end rel err was 2e-4;
bf16's 8-bit mantissa puts it at ~6e-3, inside the 2e-2 gate.

Masking is folded into the scores as an additive -100 penalty before a
single bf16 exp: masked lanes give exp(s-100) < 1e-40 which flushes to 0.0
in bf16 -- exactly the reference's masked_fill(-1e9) softmax behavior.

Layouts per core (host-side preprocessing in kernel()):
  encT  [4, E, Tx]  bf16  enc transposed  -> e on partitions (energy rhs)
  enc   [4, Tx, E]  bf16  natural         -> t on partitions (context rhs)
  w_encT [E, A], w_decT [D, A] bf16       transposed nn.Linear weights
  dec_hT [D, 4], v_col [A, 1] bf16, wb8 [128, 8] f32, mb [4, Tx] f32
"""

from contextlib import ExitStack

import numpy as np

import concourse.bass as bass
import concourse.tile as tile
from concourse import bacc, mybir
from concourse.masks import make_identity

F32 = mybir.dt.float32
BF16 = mybir.dt.bfloat16
AF = mybir.ActivationFunctionType

P = 128
N_CORES = 8
B_LOC = 4            # examples per core
TX = 1024
E = 1024             # enc_hid
A = 1024             # attn
D = 1024             # dec_hid
EO = E // P          # e-chunks
AO = A // P          # a-chunks
TO = TX // P         # t-chunks
DO = D // P          # d-chunks
NT = TX // 512       # t-tiles for energy free dim
ET = E // 512        # e-tiles for context free dim
MASK_PEN = 100.0     # additive penalty on masked scores (|s| <= ~26)


def build_nc():
    nc = bacc.Bacc(
        "TRN2", target_bir_lowering=False, debug=False, num_devices=N_CORES
    )
    encT = nc.dram_tensor("encT", [B_LOC, E, TX], BF16, kind="ExternalInput").ap()
    enc = nc.dram_tensor("enc", [B_LOC, TX, E], BF16, kind="ExternalInput").ap()
    w_encT = nc.dram_tensor("w_encT", [E, A], BF16, kind="ExternalInput").ap()
    w_decT = nc.dram_tensor("w_decT", [D, A], BF16, kind="ExternalInput").ap()
    dec_hT = nc.dram_tensor("dec_hT", [D, B_LOC], BF16, kind="ExternalInput").ap()
    v_col = nc.dram_tensor("v_col", [A, 1], BF16, kind="ExternalInput").ap()
    wb8 = nc.dram_tensor("wb8", [P, AO], F32, kind="ExternalInput").ap()
    mb = nc.dram_tensor("mb", [B_LOC, TX], F32, kind="ExternalInput").ap()
    ctx_out = nc.dram_tensor("context", [B_LOC, E], F32, kind="ExternalOutput").ap()
    alpha_out = nc.dram_tensor("alpha", [B_LOC, TX], F32, kind="ExternalOutput").ap()

    # Queue/lane discipline (each DMA-capable engine owns ONE FIFO queue):
    #   sync   (HWDGE ~200GB/s): all order-sensitive / slot-gated traffic.
    #   scalar (HWDGE ~200GB/s): ONLY dependency-free start-of-kernel loads --
    #          a gated DMA enqueue would head-of-line block the ACT compute
    #          stream (tanh/exp) and backpressure the PE.
    #   gpsimd (SWDGE ~130GB/s): small constants + the enc-natural stream.
    with tile.TileContext(nc) as tc, ExitStack() as ctx:
        const = ctx.enter_context(tc.tile_pool(name="const", bufs=1))
        big = ctx.enter_context(tc.tile_pool(name="big", bufs=6))
        en_pool = ctx.enter_context(tc.tile_pool(name="energy", bufs=6))
        small = ctx.enter_context(tc.tile_pool(name="small", bufs=2))
        rowp = ctx.enter_context(tc.tile_pool(name="rows", bufs=2))
        ep_psum = ctx.enter_context(tc.tile_pool(name="ep_ps", bufs=4, space="PSUM"))
        vec_psum = ctx.enter_context(tc.tile_pool(name="vec_ps", bufs=4, space="PSUM"))
        dram = ctx.enter_context(tc.tile_pool(name="dram", bufs=2, space="DRAM"))

        # ---- preamble loads. Bulk streams are emitted in NEED order
        # round-robin across all three DMA lanes; per-queue FIFO then acts
        # as the bandwidth arbiter. Order: w_decT a<512 (dec_proj matmuls
        # head the in-order PE stream and feed the bias chain), then the
        # w_encT / encT0-first-half pairs that gate b=0's first energy
        # group, then the encT0 second half, w_decT a>=512, encT1.
        w_encT_sb = const.tile([P, EO, A], BF16)
        encT_tiles = {}
        encT_tiles[0] = big.tile([P, EO, TX], BF16, tag="big", name="encT_sb0")
        encT_tiles[1] = big.tile([P, EO, TX], BF16, tag="big", name="encT_sb1")
        w_decT_sb = big.tile([P, DO, A], BF16, tag="big", name="w_decT_sb")

        dec_hT_sb = const.tile([P, DO, B_LOC], BF16)
        nc.gpsimd.dma_start(
            dec_hT_sb[:], dec_hT.rearrange("(do p) b -> p do b", p=P)
        )
        v_sb = const.tile([P, AO, 1], BF16)
        nc.gpsimd.dma_start(
            v_sb[:], v_col.rearrange("(ao p) one -> p ao one", p=P)
        )
        wb_sb = const.tile([P, AO], F32)
        nc.gpsimd.dma_start(wb_sb[:], wb8[:])
        mb_rows = []
        for b in range(B_LOC):
            mr = small.tile([1, TX], F32, tag="mbrow", bufs=B_LOC, name=f"mb{b}")
            nc.gpsimd.dma_start(mr[:], mb[b : b + 1, :])
            mb_rows.append(mr)
        ident4 = const.tile([B_LOC, B_LOC], F32)
        make_identity(nc, ident4[:])

        lanes = [nc.sync, nc.scalar, nc.gpsimd]
        lane_i = [0]

        def lane():
            eng = lanes[lane_i[0] % 3]
            lane_i[0] += 1
            return eng

        for do in range(DO):
            lane().dma_start(
                w_decT_sb[:, do, 0:512], w_decT[do * P : (do + 1) * P, 0:512]
            )
        for eo in range(EO):
            lane().dma_start(
                w_encT_sb[:, eo], w_encT[eo * P : (eo + 1) * P, :]
            )
            lane().dma_start(
                encT_tiles[0][:, eo, 0:512], encT[0, eo * P : (eo + 1) * P, 0:512]
            )
        for eo in range(EO):
            lane().dma_start(
                encT_tiles[0][:, eo, 512:1024],
                encT[0, eo * P : (eo + 1) * P, 512:1024],
            )
        for do in range(DO):
            lane().dma_start(
                w_decT_sb[:, do, 512:1024], w_decT[do * P : (do + 1) * P, 512:1024]
            )
        for eo in range(EO):
            lane().dma_start(
                encT_tiles[1][:, eo], encT[1, eo * P : (eo + 1) * P, :]
            )
        bias_sb = const.tile([P, AO, B_LOC], F32)

        # ---- dec_proj: psum[4b, 512a] tiles, b-stationary bf16 matmuls ----
        dp_row = rowp.tile([B_LOC, A], F32, tag="row4k", name="dp_row")

        def finalize_bias(ao_lo, ao_hi):
            # bias[a-part, b] = dec_proj^T + W_b via PE transposes (PE is
            # otherwise DMA-starved this early; no DRAM bounce needed).
            for ao in range(ao_lo, ao_hi):
                tp_ps = vec_psum.tile(
                    [P, B_LOC], F32, tag="vec", name=f"tp_ps{ao}"
                )
                nc.tensor.transpose(
                    tp_ps[:], dp_row[:, ao * P : (ao + 1) * P], ident4[:]
                )
                nc.vector.tensor_scalar_add(
                    bias_sb[:, ao], tp_ps[:], wb_sb[:, ao : ao + 1]
                )

        for at in range(A // 512):
            dp_ps = ep_psum.tile([P, 512], F32, tag="ep", name=f"dp_ps{at}")
            for do in range(DO):
                nc.tensor.matmul(
                    dp_ps[:B_LOC, :],
                    lhsT=dec_hT_sb[:, do],
                    rhs=w_decT_sb[:, do, at * 512 : (at + 1) * 512],
                    start=(do == 0),
                    stop=(do == DO - 1),
                )
            nc.vector.tensor_copy(
                dp_row[:, at * 512 : (at + 1) * 512], dp_ps[:B_LOC, :]
            )
            finalize_bias(at * 4, at * 4 + 4)

        # ---- per-example pipeline -----------------------------------------
        for b in range(B_LOC):
            if b >= 2:
                encT_tiles[b] = big.tile(
                    [P, EO, TX], BF16, tag="big", name=f"encT_sb{b}"
                )
                for eo in range(EO):
                    lane().dma_start(
                        encT_tiles[b][:, eo], encT[b, eo * P : (eo + 1) * P, :]
                    )
            encT_sb = encT_tiles[b]
            # prefetch this example's natural-layout enc for the context
            # step: one 2MB DMA, t = p*8 + to on partitions (matches the
            # expT scratch layout below)
            enc_nat = big.tile([P, TO, E], BF16, tag="big", name=f"encN_sb{b}")
            lane().dma_start(
                enc_nat[:], enc[b].rearrange("(p to) e -> p to e", p=P)
            )

            # energy^T tiles + score accumulation
            sc_ps = [
                vec_psum.tile([1, 512], F32, tag="vec", name=f"sc{b}_{nt}")
                for nt in range(NT)
            ]
            for ao in range(AO):
                for nt in range(NT):
                    ep_ps = ep_psum.tile(
                        [P, 512], F32, tag="ep", name=f"ep{b}_{ao}_{nt}"
                    )
                    for eo in range(EO):
                        nc.tensor.matmul(
                            ep_ps[:],
                            lhsT=w_encT_sb[:, eo, ao * P : (ao + 1) * P],
                            rhs=encT_sb[:, eo, nt * 512 : (nt + 1) * 512],
                            start=(eo == 0),
                            stop=(eo == EO - 1),
                        )
                    energy = en_pool.tile(
                        [P, 512], BF16, tag="energy", name=f"en{b}_{ao}_{nt}"
                    )
                    nc.scalar.activation(
                        energy[:], ep_ps[:], AF.Tanh, bias=bias_sb[:, ao, b : b + 1]
                    )
                    nc.tensor.matmul(
                        sc_ps[nt][:],
                        lhsT=v_sb[:, ao],
                        rhs=energy[:],
                        start=(ao == 0),
                        stop=(ao == AO - 1),
                    )

            # masked softmax. Scores are bounded (|s| <= sum|v| ~ 26 since
            # |tanh| <= 1) so exp needs no max shift -- softmax is
            # shift-invariant. Mask is applied as an additive -100 on the
            # scores; exp then flushes masked lanes to 0.0 in bf16. The
            # bf16 exp row round-trips through DRAM to land t on
            # partitions for the context matmul (sbuf->sbuf
            # partition-crossing reshapes don't balance as DMA APs).
            exp_bf = rowp.tile([1, TX], BF16, tag="erow", name=f"exp{b}")
            exp_scr = dram.tile([TX], BF16, tag="escr", name=f"escr{b}")
            expT = small.tile([P, TO], BF16, tag="expT", name=f"expT{b}")
            exp_scr_t = exp_scr.rearrange("(p to) -> p to", p=P)
            HP = P // NT
            for nt in range(NT):
                hs = slice(nt * 512, (nt + 1) * 512)
                sc_m = small.tile(
                    [1, 512], F32, tag="scm", bufs=NT, name=f"scm{b}_{nt}"
                )
                nc.vector.tensor_add(
                    out=sc_m[:], in0=sc_ps[nt][:], in1=mb_rows[b][:, hs]
                )
                nc.scalar.activation(exp_bf[:, hs], sc_m[:], AF.Exp)
                nc.sync.dma_start(exp_scr[None, hs], exp_bf[:, hs])
                # t = p*8 + to: score half nt covers partitions [nt*64, ...)
                nc.sync.dma_start(
                    expT[nt * HP : (nt + 1) * HP, :],
                    exp_scr_t[nt * HP : (nt + 1) * HP, :],
                )

            ssum = small.tile([1, 1], F32, tag="ssum", name=f"ssum{b}")
            nc.vector.reduce_sum(ssum[:], exp_bf[:], axis=mybir.AxisListType.X)
            rsum = small.tile([1, 1], F32, tag="rsum", name=f"rsum{b}")
            nc.vector.reciprocal(rsum[:], ssum[:])
            alpha_row = rowp.tile([1, TX], F32, tag="arow", name=f"alpha{b}")
            nc.vector.tensor_scalar_mul(alpha_row[:], exp_bf[:], rsum[:])
            nc.sync.dma_start(alpha_out[b : b + 1, :], alpha_row[:])

            # context[e] = sum_t alpha[t] enc[t, e]; normalization by 1/sum
            # is folded into the PSUM evacuation
            ctx_row = rowp.tile([1, E], F32, tag="row4k", name=f"ctx{b}")
            for et in range(ET):
                cx_ps = vec_psum.tile([1, 512], F32, tag="vec", name=f"cx{b}_{et}")
                for to in range(TO):
                    nc.tensor.matmul(
                        cx_ps[:],
                        lhsT=expT[:, to : to + 1],
                        rhs=enc_nat[:, to, et * 512 : (et + 1) * 512],
                        start=(to == 0),
                        stop=(to == TO - 1),
                    )
                nc.vector.tensor_scalar_mul(
                    ctx_row[:, et * 512 : (et + 1) * 512], cx_ps[:], rsum[:]
                )
            nc.sync.dma_start(ctx_out[b : b + 1, :], ctx_row[:])

    nc.compile()
    return nc


_NC = None


def _get_nc():
    global _NC
    if _NC is None:
        _NC = build_nc()
    return _NC


def make_in_maps(dec_hidden, enc_outputs, mask, W_w, W_b, v_w):
    import ml_dtypes

    BF = ml_dtypes.bfloat16
    dec_hidden = np.asarray(dec_hidden, np.float32)
    enc_outputs = np.asarray(enc_outputs, np.float32)
    W_w = np.asarray(W_w, np.float32)
    W_b = np.asarray(W_b, np.float32)
    v_w = np.asarray(v_w, np.float32)
    mb = (np.asarray(mask).astype(np.float32) - 1.0) * MASK_PEN

    enc = np.ascontiguousarray(enc_outputs.astype(BF))
    encT = np.ascontiguousarray(enc.transpose(0, 2, 1))
    w_encT = np.ascontiguousarray(W_w[:, D:].T.astype(BF))
    w_decT = np.ascontiguousarray(W_w[:, :D].T.astype(BF))
    wb8 = np.ascontiguousarray(W_b.reshape(AO, P).T)
    v_col = np.ascontiguousarray(v_w.reshape(A, 1).astype(BF))

    in_maps = []
    for c in range(N_CORES):
        sl = slice(B_LOC * c, B_LOC * (c + 1))
        in_maps.append(
            {
                "encT": encT[sl],
                "enc": enc[sl],
                "w_encT": w_encT,
                "w_decT": w_decT,
                "dec_hT": np.ascontiguousarray(dec_hidden[sl].T.astype(BF)),
                "v_col": v_col,
                "wb8": wb8,
                "mb": np.ascontiguousarray(mb[sl]),
            }
        )
    return in_maps


def kernel(dec_hidden, enc_outputs, mask, W_w, W_b, v_w):
    from concourse.bass_utils import run_bass_kernel_spmd

    assert enc_outputs.shape == (N_CORES * B_LOC, TX, E), enc_outputs.shape
    nc = _get_nc()
    in_maps = make_in_maps(dec_hidden, enc_outputs, mask, W_w, W_b, v_w)
    res = run_bass_kernel_spmd(nc, in_maps, list(range(N_CORES))).results
    context = np.concatenate([res[c]["context"] for c in range(N_CORES)], axis=0)
    alpha = np.concatenate([res[c]["alpha"] for c in range(N_CORES)], axis=0)
    return context, alpha


# revision 9
# speedup vs baseline: 1.0634x; 1.0187x over previous
"""Bahdanau attention kernel for Trainium2 (Bass/Tile), 8-core data-parallel.

Problem shapes: B=32, Tx=1024, enc_hid=dec_hid=attn=1024, fp32 in/out.

Math (per example b):
  dec_proj = W_dec @ dec_hidden[b]                 [attn]
  energy^T[a, t] = tanh(sum_e W_enc[a,e] enc[b,t,e] + dec_proj[a] + W_b[a])
  scores[t] = sum_a v[a] energy^T[a, t]
  alpha = softmax(mask(scores))
  context[e] = sum_t alpha[t] enc[b,t,e]

Sharding: batch B split 4 examples per core across 8 cores; weights replicated.

All matmul operands are bf16: on TRN2 silicon a 512-moving-row fp32r matmul
measures ~394ns while bf16 measures ~216ns (1 col/cycle @2.4GHz), and bf16
halves HBM traffic (42MB -> 21MB per core) and enables fast weight load.
PSUM accumulation stays fp32. Measured fp22 end-to-end rel err was 2e-4;
bf16's 8-bit mantissa puts it at ~6e-3, inside the 2e-2 gate.

Masking is folded into the scores as an additive -100 penalty before a
single bf16 exp: masked lanes give exp(s-100) < 1e-40 which flushes to 0.0
in bf16 -- exactly the reference's masked_fill(-1e9) softmax behavior.

Layouts per core (host-side preprocessing in kernel()):
  encT  [4, E, Tx]  bf16  enc transposed  -> e on partitions (energy rhs)
  enc   [4, Tx, E]  bf16  natural         -> t on partitions (context rhs)
  w_encT [E, A], w_decT [D, A] bf16       transposed nn.Linear weights
  dec_hT [D, 4], v_col [A, 1] bf16, wb8 [128, 8] f32, mb [4, Tx] f32
"""

from contextlib import ExitStack

import numpy as np

import concourse.bass as bass
import concourse.tile as tile
from concourse import bacc, mybir
from concourse.masks import make_identity

F32 = mybir.dt.float32
BF16 = mybir.dt.bfloat16
AF = mybir.ActivationFunctionType

P = 128
N_CORES = 8
B_LOC = 4            # examples per core
TX = 1024
E = 1024             # enc_hid
A = 1024             # attn
D = 1024             # dec_hid
EO = E // P          # e-chunks
AO = A // P          # a-chunks
TO = TX // P         # t-chunks
DO = D // P          # d-chunks
NT = TX // 512       # t-tiles for energy free dim
ET = E // 512        # e-tiles for context free dim
MASK_PEN = 100.0     # additive penalty on masked scores (|s| <= ~26)


def build_nc():
    nc = bacc.Bacc(
        "TRN2", target_bir_lowering=False, debug=False, num_devices=N_CORES
    )
    encT = nc.dram_tensor("encT", [B_LOC, E, TX], BF16, kind="ExternalInput").ap()
    enc = nc.dram_tensor("enc", [B_LOC, TX, E], BF16, kind="ExternalInput").ap()
    w_encT = nc.dram_tensor("w_encT", [E, A], BF16, kind="ExternalInput").ap()
    w_decT = nc.dram_tensor("w_decT", [D, A], BF16, kind="ExternalInput").ap()
    dec_hT = nc.dram_tensor("dec_hT", [D, B_LOC], BF16, kind="ExternalInput").ap()
    v_col = nc.dram_tensor("v_col", [A, 1], BF16, kind="ExternalInput").ap()
    wb8 = nc.dram_tensor("wb8", [P, AO], F32, kind="ExternalInput").ap()
    mb = nc.dram_tensor("mb", [B_LOC, TX], F32, kind="ExternalInput").ap()
    ctx_out = nc.dram_tensor("context", [B_LOC, E], F32, kind="ExternalOutput").ap()
    alpha_out = nc.dram_tensor("alpha", [B_LOC, TX], F32, kind="ExternalOutput").ap()

    # Queue/lane discipline (each DMA-capable engine owns ONE FIFO queue):
    #   sync   (HWDGE ~200GB/s): all order-sensitive / slot-gated traffic.
    #   scalar (HWDGE ~200GB/s): ONLY dependency-free start-of-kernel loads --
    #          a gated DMA enqueue would head-of-line block the ACT compute
    #          stream (tanh/exp) and backpressure the PE.
    #   gpsimd (SWDGE ~130GB/s): small constants + the enc-natural stream.
    with tile.TileContext(nc) as tc, ExitStack() as ctx:
        const = ctx.enter_context(tc.tile_pool(name="const", bufs=1))
        big = ctx.enter_context(tc.tile_pool(name="big", bufs=6))
        en_pool = ctx.enter_context(tc.tile_pool(name="energy", bufs=6))
        small = ctx.enter_context(tc.tile_pool(name="small", bufs=2))
        rowp = ctx.enter_context(tc.tile_pool(name="rows", bufs=2))
        ep_psum = ctx.enter_context(tc.tile_pool(name="ep_ps", bufs=4, space="PSUM"))
        vec_psum = ctx.enter_context(tc.tile_pool(name="vec_ps", bufs=4, space="PSUM"))
        dram = ctx.enter_context(tc.tile_pool(name="dram", bufs=2, space="DRAM"))

        # ---- preamble loads. Bulk streams are emitted in NEED order
        # round-robin across all three DMA lanes; per-queue FIFO then acts
        # as the bandwidth arbiter. Order: w_decT a<512 (dec_proj matmuls
        # head the in-order PE stream and feed the bias chain), then the
        # w_encT / encT0-first-half pairs that gate b=0's first energy
        # group, then the encT0 second half, w_decT a>=512, encT1.
        w_encT_sb = const.tile([P, EO, A], BF16)
        encT_tiles = {}
        encT_tiles[0] = big.tile([P, EO, TX], BF16, tag="big", name="encT_sb0")
        encT_tiles[1] = big.tile([P, EO, TX], BF16, tag="big", name="encT_sb1")
        w_decT_sb = big.tile([P, DO, A], BF16, tag="big", name="w_decT_sb")

        dec_hT_sb = const.tile([P, DO, B_LOC], BF16)
        nc.gpsimd.dma_start(
            dec_hT_sb[:], dec_hT.rearrange("(do p) b -> p do b", p=P)
        )
        v_sb = const.tile([P, AO, 1], BF16)
        nc.gpsimd.dma_start(
            v_sb[:], v_col.rearrange("(ao p) one -> p ao one", p=P)
        )
        wb_sb = const.tile([P, AO], F32)
        nc.gpsimd.dma_start(wb_sb[:], wb8[:])
        mb_rows = []
        for b in range(B_LOC):
            mr = small.tile([1, TX], F32, tag="mbrow", bufs=B_LOC, name=f"mb{b}")
            nc.gpsimd.dma_start(mr[:], mb[b : b + 1, :])
            mb_rows.append(mr)
        ident4 = const.tile([B_LOC, B_LOC], F32)
        make_identity(nc, ident4[:])

        lanes = [nc.sync, nc.scalar, nc.gpsimd]
        lane_i = [0]

        def lane():
            eng = lanes[lane_i[0] % 3]
            lane_i[0] += 1
            return eng

        for do in range(DO):
            lane().dma_start(
                w_decT_sb[:, do, 0:512], w_decT[do * P : (do + 1) * P, 0:512]
            )
        for eo in range(EO):
            lane().dma_start(
                w_encT_sb[:, eo], w_encT[eo * P : (eo + 1) * P, :]
            )
            lane().dma_start(
                encT_tiles[0][:, eo], encT[0, eo * P : (eo + 1) * P, :]
            )
        for do in range(DO):
            lane().dma_start(
                w_decT_sb[:, do, 512:1024], w_decT[do * P : (do + 1) * P, 512:1024]
            )
        for eo in range(EO):
            lane().dma_start(
                encT_tiles[1][:, eo], encT[1, eo * P : (eo + 1) * P, :]
            )
        bias_sb = const.tile([P, AO, B_LOC], F32)

        # ---- dec_proj: psum[4b, 512a] tiles, b-stationary bf16 matmuls ----
        dp_row = rowp.tile([B_LOC, A], F32, tag="row4k", name="dp_row")

        def finalize_bias(ao_lo, ao_hi):
            # bias[a-part, b] = dec_proj^T + W_b via PE transposes (PE is
            # otherwise DMA-starved this early; no DRAM bounce needed).
            for ao in range(ao_lo, ao_hi):
                tp_ps = vec_psum.tile(
                    [P, B_LOC], F32, tag="vec", name=f"tp_ps{ao}"
                )
                nc.tensor.transpose(
                    tp_ps[:], dp_row[:, ao * P : (ao + 1) * P], ident4[:]
                )
                nc.vector.tensor_scalar_add(
                    bias_sb[:, ao], tp_ps[:], wb_sb[:, ao : ao + 1]
                )

        def dec_proj_pass(at):
            dp_ps = ep_psum.tile([P, 512], F32, tag="ep", name=f"dp_ps{at}")
            for do in range(DO):
                nc.tensor.matmul(
                    dp_ps[:B_LOC, :],
                    lhsT=dec_hT_sb[:, do],
                    rhs=w_decT_sb[:, do, at * 512 : (at + 1) * 512],
                    start=(do == 0),
                    stop=(do == DO - 1),
                )
            nc.vector.tensor_copy(
                dp_row[:, at * 512 : (at + 1) * 512], dp_ps[:B_LOC, :]
            )
            finalize_bias(at * 4, at * 4 + 4)

        # at=0 runs at the head of the PE stream (its w_decT half leads the
        # DMA order); at=1's weights arrive after w_encT+encT0, so that pass
        # is deferred into b=0's energy stream to avoid head-of-line
        # blocking the in-order PE queue.
        dec_proj_pass(0)

        # ---- per-example pipeline -----------------------------------------
        # The PE queue is in-order, so every matmul that waits on a
        # non-PE producer is emitted at least one ao-pass after that
        # producer's input was ready: score matmuls trail their tanh by one
        # pass, and example b's context matmuls are emitted inside example
        # b+1's energy stream (giving the exp -> DRAM -> expT bounce time
        # to land).
        pend_score = None   # (ao, [energy tiles per nt]) awaiting score MMs
        pend_ctx = None     # closure emitting the previous example's context

        for b in range(B_LOC):
            if b >= 2:
                encT_tiles[b] = big.tile(
                    [P, EO, TX], BF16, tag="big", name=f"encT_sb{b}"
                )
                for eo in range(EO):
                    lane().dma_start(
                        encT_tiles[b][:, eo], encT[b, eo * P : (eo + 1) * P, :]
                    )
            encT_sb = encT_tiles[b]
            # prefetch this example's natural-layout enc for the context
            # step: one 2MB DMA, t = p*8 + to on partitions (matches the
            # expT scratch layout below)
            enc_nat = big.tile([P, TO, E], BF16, tag="big", name=f"encN_sb{b}")
            lane().dma_start(
                enc_nat[:], enc[b].rearrange("(p to) e -> p to e", p=P)
            )

            sc_ps = [
                vec_psum.tile([1, 512], F32, tag="vec", name=f"sc{b}_{nt}")
                for nt in range(NT)
            ]

            # energy^T tiles + deferred score accumulation. eo-outer /
            # nt-inner so both nt-halves share each weight load.
            for ao in range(AO):
                eps = [
                    ep_psum.tile([P, 512], F32, tag="ep", name=f"ep{b}_{ao}_{nt}")
                    for nt in range(NT)
                ]
                for eo in range(EO):
                    for nt in range(NT):
                        nc.tensor.matmul(
                            eps[nt][:],
                            lhsT=w_encT_sb[:, eo, ao * P : (ao + 1) * P],
                            rhs=encT_sb[:, eo, nt * 512 : (nt + 1) * 512],
                            start=(eo == 0),
                            stop=(eo == EO - 1),
                        )
                ens = []
                for nt in range(NT):
                    energy = en_pool.tile(
                        [P, 512], BF16, tag="energy", name=f"en{b}_{ao}_{nt}"
                    )
                    nc.scalar.activation(
                        energy[:], eps[nt][:], AF.Tanh,
                        bias=bias_sb[:, ao, b : b + 1],
                    )
                    ens.append(energy)
                if pend_score is not None:
                    pend_score()

                def flush_score(ao=ao, ens=ens, sc_ps=sc_ps):
                    for nt in range(NT):
                        nc.tensor.matmul(
                            sc_ps[nt][:],
                            lhsT=v_sb[:, ao],
                            rhs=ens[nt][:],
                            start=(ao == 0),
                            stop=(ao == AO - 1),
                        )

                pend_score = flush_score
                if b == 0 and ao == 1:
                    dec_proj_pass(1)
                if pend_ctx is not None and ao == 1:
                    pend_ctx()
                    pend_ctx = None
            pend_score()
            pend_score = None

            # masked softmax. Scores are bounded (|s| <= sum|v| ~ 26 since
            # |tanh| <= 1) so exp needs no max shift -- softmax is
            # shift-invariant. Mask is applied as an additive -100 on the
            # scores; exp then flushes masked lanes to 0.0 in bf16. The
            # bf16 exp row round-trips through DRAM to land t on
            # partitions for the context matmul (sbuf->sbuf
            # partition-crossing reshapes don't balance as DMA APs).
            exp_bf = rowp.tile([1, TX], BF16, tag="erow", name=f"exp{b}")
            exp_scr = dram.tile([TX], BF16, tag="escr", name=f"escr{b}")
            expT = small.tile([P, TO], BF16, tag="expT", name=f"expT{b}")
            exp_scr_t = exp_scr.rearrange("(p to) -> p to", p=P)
            HP = P // NT
            for nt in range(NT):
                hs = slice(nt * 512, (nt + 1) * 512)
                sc_m = small.tile(
                    [1, 512], F32, tag="scm", bufs=NT, name=f"scm{b}_{nt}"
                )
                nc.vector.tensor_add(
                    out=sc_m[:], in0=sc_ps[nt][:], in1=mb_rows[b][:, hs]
                )
                nc.scalar.activation(exp_bf[:, hs], sc_m[:], AF.Exp)
                nc.sync.dma_start(exp_scr[None, hs], exp_bf[:, hs])
                # t = p*8 + to: score half nt covers partitions [nt*64, ...)
                nc.sync.dma_start(
                    expT[nt * HP : (nt + 1) * HP, :],
                    exp_scr_t[nt * HP : (nt + 1) * HP, :],
                )

            ssum = small.tile([1, 1], F32, tag="ssum", name=f"ssum{b}")
            nc.vector.reduce_sum(ssum[:], exp_bf[:], axis=mybir.AxisListType.X)
            rsum = small.tile([1, 1], F32, tag="rsum", name=f"rsum{b}")
            nc.vector.reciprocal(rsum[:], ssum[:])
            alpha_row = rowp.tile([1, TX], F32, tag="arow", name=f"alpha{b}")
            nc.vector.tensor_scalar_mul(alpha_row[:], exp_bf[:], rsum[:])
            nc.sync.dma_start(alpha_out[b : b + 1, :], alpha_row[:])

            def emit_ctx(b=b, expT=expT, enc_nat=enc_nat, rsum=rsum):
                # context[e] = sum_t alpha[t] enc[t, e]; normalization by
                # 1/sum is folded into the PSUM evacuation. to-outer /
                # et-inner so both et-halves share each expT column load.
                ctx_row = rowp.tile([1, E], F32, tag="row4k", name=f"ctx{b}")
                cxs = [
                    vec_psum.tile([1, 512], F32, tag="vec", name=f"cx{b}_{et}")
                    for et in range(ET)
                ]
                for to in range(TO):
                    for et in range(ET):
                        nc.tensor.matmul(
                            cxs[et][:],
                            lhsT=expT[:, to : to + 1],
                            rhs=enc_nat[:, to, et * 512 : (et + 1) * 512],
                            start=(to == 0),
                            stop=(to == TO - 1),
                        )
                for et in range(ET):
                    nc.vector.tensor_scalar_mul(
                        ctx_row[:, et * 512 : (et + 1) * 512], cxs[et][:], rsum[:]
                    )
                nc.sync.dma_start(ctx_out[b : b + 1, :], ctx_row[:])

            pend_ctx = emit_ctx

        pend_ctx()

    nc.compile()
    return nc


_NC = None


def _get_nc():
    global _NC
    if _NC is None:
        _NC = build_nc()
    return _NC


def make_in_maps(dec_hidden, enc_outputs, mask, W_w, W_b, v_w):
    import ml_dtypes

    BF = ml_dtypes.bfloat16
    dec_hidden = np.asarray(dec_hidden, np.float32)
    enc_outputs = np.asarray(enc_outputs, np.float32)
    W_w = np.asarray(W_w, np.float32)
    W_b = np.asarray(W_b, np.float32)
    v_w = np.asarray(v_w, np.float32)
    mb = (np.asarray(mask).astype(np.float32) - 1.0) * MASK_PEN

    enc = np.ascontiguousarray(enc_outputs.astype(BF))
    encT = np.ascontiguousarray(enc.transpose(0, 2, 1))
    w_encT = np.ascontiguousarray(W_w[:, D:].T.astype(BF))
    w_decT = np.ascontiguousarray(W_w[:, :D].T.astype(BF))
    wb8 = np.ascontiguousarray(W_b.reshape(AO, P).T)
    v_col = np.ascontiguousarray(v_w.reshape(A, 1).astype(BF))

    in_maps = []
    for c in range(N_CORES):
        sl = slice(B_LOC * c, B_LOC * (c + 1))
        in_maps.append(
            {
                "encT": encT[sl],
                "enc": enc[sl],
                "w_encT": w_encT,
                "w_decT": w_decT,
                "dec_hT": np.ascontiguousarray(dec_hidden[sl].T.astype(BF)),
                "v_col": v_col,
                "wb8": wb8,
                "mb": np.ascontiguousarray(mb[sl]),
            }
        )
    return in_maps


def kernel(dec_hidden, enc_outputs, mask, W_w, W_b, v_w):
    from concourse.bass_utils import run_bass_kernel_spmd

    assert enc_outputs.shape == (N_CORES * B_LOC, TX, E), enc_outputs.shape
    nc = _get_nc()
    in_maps = make_in_maps(dec_hidden, enc_outputs, mask, W_w, W_b, v_w)
    res = run_bass_kernel_spmd(nc, in_maps, list(range(N_CORES))).results
    context = np.concatenate([res[c]["context"] for c in range(N_CORES)], axis=0)
    alpha = np.concatenate([res[c]["alpha"] for c in range(N_CORES)], axis=0)
    return context, alpha
